# revision 28
# baseline (speedup 1.0000x reference)
"""Trainium2 Bass kernel for the CP-sparse-degree-LU module.

Reference computation (all fp32):
    zf  = z.reshape(-1, 2048)                      # [N=8192, d]
    W   = masks * U                                # [6, k, d]
    out = zf @ W[0].T                              # [N, k]
    for i in 1..5: out = (zf @ W[i].T) * out + out
    x   = out @ C_w.T + C_b                        # [N, o]

Sharding: data-parallel over the token dim N across 8 cores (1024 tokens
each), weights replicated; no collectives. Everything is laid out
transposed on device (acc is [k, tok], output is [o, tok]) so the degree
chain and the final projection both run without on-device transposes:
    acc.T = W_i @ z.T  -> lhsT = W_i.T tiles [d,k], rhs = z.T [d, tok]
    x.T   = C_w @ acc  -> lhsT = C_w.T tiles [k,o], rhs = acc [k, tok]

Sparsity: W = masks*U is block-sparse (tril/triu factors plus a degree
mask that zeroes rank rows < i*K/DEGREE at degree i). The host detects
all-zero 128x128 blocks of the actual W at runtime and builds the device
program skipping them: a skipped (degree, rank-tile) group contributes
mm = 0, so acc = (0+1)*acc is the identity and the whole group (DMA,
matmuls, DVE update) is dropped. This is sound for arbitrary inputs —
only provably-zero blocks are skipped; dense inputs yield the dense
program.

Matmuls run in float32r (fp32 bits read FP22-truncated, single-pass full
rate vs 4-pass true fp32); the chain update acc = (mm + 1) * acc is one
DVE scalar_tensor_tensor op reading the PSUM accumulation directly.

The matmul stream is the roofline (741 128x128-blocks x 1024 tokens at
1 row/cycle, 2.4 GHz ~= 316 us); everything else here exists to keep
that stream dense:
  - a burst of dummy matmuls on scratch SBUF ramps the PE p-state
    (0.65 -> 2.4 GHz takes ~3 us of busy) while the first DMAs land;
  - z.T is loaded in [128, 512] chunks so the first real matmul only
    waits on 256 KB, and chunk tiles stream in consumption order;
  - PSUM tiles are single-bank [128, 512] (8 in flight) so accumulation
    groups free banks as soon as their chunk is consumed;
  - degree-0 PSUM->acc copies run on the (otherwise idle) Scalar engine,
    leaving the DVE for the chain updates; the final bias-adds alternate
    Scalar/Vector so the last output tile drains on both engines.
"""

import os
import sys
import types
from contextlib import ExitStack

import numpy as np

DEGREE, D, K, O = 6, 2048, 2048, 2048
N_CORES = 8
N_TOTAL = 8192
TOK = N_TOTAL // N_CORES  # 1024 tokens per core
P = 128
DT = D // P  # 16 contraction tiles (degree matmuls)
KT = K // P  # 16 rank tiles
OT = O // P  # 16 output tiles
NC = 512  # moving free dim per matmul (one PSUM bank of fp32)
TC = TOK // NC  # 2 token chunks
NDUMMY = 12  # PE p-state pre-ramp matmuls (cover the initial DMA wait)
ZCHUNKED = 2  # leading z tiles split into [P, NC] chunks for a fast start

_CACHE = {}


def _install_ntff_shim():
    """Register antenv.axon_hooks so run_bass_kernel_spmd(trace=True) can
    profile under axon. Safe no-op if anything is unavailable."""
    try:
        if "antenv.axon_hooks" in sys.modules:
            return
        mod = types.ModuleType("antenv.axon_hooks")
        mod._hook = None
        mod.set_axon_ntff_profile_hook = lambda h: setattr(mod, "_hook", h)
        mod.get_axon_ntff_profile_hook = lambda: mod._hook
        sys.modules["antenv.axon_hooks"] = mod
        from trn_agent_boot.trn_boot import _ntff_profile_via_ctypes

        mod._hook = _ntff_profile_via_ctypes("/opt/axon/libaxon_pjrt.so")
    except Exception:
        pass


def _build(ranges):
    """ranges[i][kt] = (dt_lo, dt_hi) inclusive active range, or None if the
    whole (degree, rank-tile) block row is zero."""
    import concourse.tile as tile
    from concourse import bacc, mybir

    f32 = mybir.dt.float32
    f32r = mybir.dt.float32r
    bf16 = mybir.dt.bfloat16
    ADD = mybir.AluOpType.add
    MULT = mybir.AluOpType.mult

    nc = bacc.Bacc("TRN2", target_bir_lowering=False, debug=False)

    # z.T per core, tiled: [di, dt*TOK + t] = z[t, dt*P + di]
    z_d = nc.dram_tensor("z", [P, DT * TOK], bf16, kind="ExternalInput")
    # W per degree/rank-tile: [i, kt, di, dt*P + ki] = W[i, kt*P+ki, dt*P+di]
    w_d = nc.dram_tensor("w", [DEGREE, KT, P, DT * P], bf16, kind="ExternalInput")
    # C_w tiled: [ot, ki, kt*P + oi] = C_w[ot*P+oi, kt*P+ki]
    c_d = nc.dram_tensor("c", [OT, P, KT * P], bf16, kind="ExternalInput")
    # C_b tiled: [oi, ot] = C_b[ot*P + oi]
    cb_d = nc.dram_tensor("cb", [P, OT], f32, kind="ExternalInput")
    # x.T: [o, t]
    x_d = nc.dram_tensor("x", [O, TOK], f32, kind="ExternalOutput")

    z_ap, w_ap, c_ap, cb_ap, x_ap = (t.ap() for t in (z_d, w_d, c_d, cb_d, x_d))

    with tile.TileContext(nc) as tc, ExitStack() as ctx:
        zcpool = ctx.enter_context(tc.tile_pool(name="zc", bufs=ZCHUNKED * TC))
        zpool = ctx.enter_context(tc.tile_pool(name="z", bufs=DT - ZCHUNKED))
        accpool = ctx.enter_context(tc.tile_pool(name="acc", bufs=KT))
        wpool = ctx.enter_context(tc.tile_pool(name="w", bufs=10))
        cbpool = ctx.enter_context(tc.tile_pool(name="cb", bufs=1))
        xpool = ctx.enter_context(tc.tile_pool(name="xt", bufs=4))
        scrpool = ctx.enter_context(tc.tile_pool(name="scr", bufs=1))
        pspool = ctx.enter_context(tc.tile_pool(name="ps", bufs=8, space="PSUM"))

        # Resident z.T buffers: the first ZCHUNKED tiles are split into
        # [128, NC] chunks (fine-grained deps -> the first matmul starts as
        # soon as 256 KB lands); the rest are whole [128, TOK] tiles (DMA
        # queue issue slots cost ~0.6 us each, so fewer is better).
        z_sb = []
        for j in range(DT):
            if j < ZCHUNKED:
                z_sb.append(
                    [
                        zcpool.tile([P, NC], bf16, tag="zc", name=f"z_sb{j}_{c}")
                        for c in range(TC)
                    ]
                )
            else:
                z_sb.append(zpool.tile([P, TOK], bf16, tag="z", name=f"z_sb{j}"))
        acc = [accpool.tile([P, TOK], bf16, tag="acc", name=f"acc{j}") for j in range(KT)]
        cb_sb = cbpool.tile([P, OT], f32)

        # --- PE p-state pre-ramp -------------------------------------
        # The PE clock ramps 0.65 -> 2.4 GHz over ~3 us of continuous
        # execution. Burn the initial DMA wait on dummy matmuls over a
        # memset scratch tile so the real stream starts at full clock.
        scr = scrpool.tile([P, 2 * P + 1], bf16, tag="scr")
        # The Vector queue is empty at program start, so the dummies'
        # dependency clears early. Small moving dim (128) keeps each dummy
        # short: the real stream starts at most one dummy after its data
        # lands, while the burst still pre-ramps the PE clock.
        nc.vector.memset(scr[:], 0.0)
        ps_warm = pspool.tile([P, NC], f32, tag="ps")
        for r in range(NDUMMY):
            nc.tensor.matmul(
                ps_warm[:, :P],
                scr[:, :P],
                scr[:, P : 2 * P],
                start=True,
                stop=True,
            )
        # Tile framework requires a reader for every written tile.
        nc.scalar.copy(scr[:, 2 * P : 2 * P + 1], ps_warm[:, :1])

        # Issue each z DMA lazily, right before the first group that reads
        # it — with the tril structure of degree 0 this streams z in as the
        # early rank-tile groups consume it. z rides the GpSimd queue and
        # the weights ride Sync, each in exact consumption order: the two
        # transfers needed next (z_k and w_k) then proceed concurrently.
        z_issued = [[False] * TC for _ in range(DT)]

        def ensure_z(lo_, hi_, chunks=(0, 1)):
            for dt_ in range(lo_, hi_ + 1):
                if dt_ < ZCHUNKED:
                    for c_ in chunks:
                        if not z_issued[dt_][c_]:
                            nc.gpsimd.dma_start(
                                z_sb[dt_][c_][:],
                                z_ap[
                                    :,
                                    dt_ * TOK + c_ * NC : dt_ * TOK + (c_ + 1) * NC,
                                ],
                            )
                            z_issued[dt_][c_] = True
                elif not z_issued[dt_][0]:
                    nc.gpsimd.dma_start(
                        z_sb[dt_][:], z_ap[:, dt_ * TOK : (dt_ + 1) * TOK]
                    )
                    z_issued[dt_][0] = True

        # Degree chain over acc[kt-block, tokens].
        for i in range(DEGREE):
            for kt in range(KT):
                rng = ranges[i][kt]
                if rng is None:
                    if i == 0:
                        # acc = mm = 0 for this rank block
                        nc.gpsimd.memset(acc[kt][:], 0.0)
                    continue
                lo, hi = rng
                ndt = hi - lo + 1
                w_sb = wpool.tile([P, ndt * P], bf16, tag="w")
                if i == 0:
                    # Degree 0 streams z in. Issue in consumption order:
                    # the chunk-0 z parts (needed with w for the first
                    # matmul), then the small w tile, then chunk-1 parts.
                    ensure_z(lo, hi, chunks=(0,))
                    nc.sync.dma_start(w_sb[:], w_ap[i, kt][:, lo * P : (hi + 1) * P])
                    ensure_z(lo, hi, chunks=(1,))
                else:
                    nc.sync.dma_start(w_sb[:], w_ap[i, kt][:, lo * P : (hi + 1) * P])
                    ensure_z(lo, hi)
                for tcx in range(TC):
                    ps = pspool.tile([P, NC], f32, tag="ps")
                    for j, dt in enumerate(range(lo, hi + 1)):
                        if dt < ZCHUNKED:
                            zmv = z_sb[dt][tcx][:]
                        else:
                            zmv = z_sb[dt][:, tcx * NC : (tcx + 1) * NC]
                        nc.tensor.matmul(
                            ps[:],
                            w_sb[:, j * P : (j + 1) * P],
                            zmv,
                            start=(j == 0),
                            stop=(j == ndt - 1),
                        )
                    sl = slice(tcx * NC, (tcx + 1) * NC)
                    if i == 0:
                        # PSUM recycling gates the small early groups, so
                        # split the PSUM -> acc copies across the Scalar and
                        # Vector engines for 2x drain throughput.
                        if tcx == 0:
                            nc.scalar.copy(acc[kt][:, sl], ps[:])
                        else:
                            nc.vector.tensor_copy(acc[kt][:, sl], ps[:])
                    else:
                        # acc = (mm + 1) * acc  — one DVE op
                        nc.vector.scalar_tensor_tensor(
                            acc[kt][:, sl], ps[:], 1.0, acc[kt][:, sl], ADD, MULT
                        )

        # Final projection: x.T[ot-block] = C_w @ acc + C_b
        nc.sync.dma_start(cb_sb[:], cb_ap)
        for ot in range(OT):
            c_sb = wpool.tile([P, KT * P], bf16, tag="w")
            nc.sync.dma_start(c_sb[:], c_ap[ot])
            for tcx in range(TC):
                ps = pspool.tile([P, NC], f32, tag="ps")
                for kt in range(KT):
                    nc.tensor.matmul(
                        ps[:],
                        c_sb[:, kt * P : (kt + 1) * P],
                        acc[kt][:, tcx * NC : (tcx + 1) * NC],
                        start=(kt == 0),
                        stop=(kt == KT - 1),
                    )
                xt = xpool.tile([P, NC], f32, tag="xt")
                if ot == OT - 1 and tcx == TC - 1:
                    # Final chunk: split in half, bias on Scalar and Vector
                    # in parallel, stores on queues with no instruction
                    # queued ahead, so the tail after the last matmul is
                    # just one bias + one small store.
                    half = NC // 2
                    nc.vector.tensor_scalar_add(
                        xt[:, :half], ps[:, :half], cb_sb[:, ot : ot + 1]
                    )
                    nc.sync.dma_start(
                        x_ap[ot * P : (ot + 1) * P, tcx * NC : tcx * NC + half],
                        xt[:, :half],
                    )
                    nc.scalar.add(
                        xt[:, half:], ps[:, half:], cb_sb[:, ot : ot + 1]
                    )
                    nc.gpsimd.dma_start(
                        x_ap[ot * P : (ot + 1) * P, tcx * NC + half : (tcx + 1) * NC],
                        xt[:, half:],
                    )
                else:
                    osl = slice(tcx * NC, (tcx + 1) * NC)
                    # Alternate bias-adds between Scalar and Vector so the
                    # chunks drain on both engines in parallel; spread the
                    # stores over two queues to halve each queue's backlog.
                    if tcx % 2 == 0:
                        nc.scalar.add(xt[:], ps[:], cb_sb[:, ot : ot + 1])
                        nc.gpsimd.dma_start(x_ap[ot * P : (ot + 1) * P, osl], xt[:])
                    else:
                        nc.vector.tensor_scalar_add(
                            xt[:], ps[:], cb_sb[:, ot : ot + 1]
                        )
                        nc.scalar.dma_start(x_ap[ot * P : (ot + 1) * P, osl], xt[:])

    nc.compile()
    return nc


def kernel(z, U, masks, C_w, C_b):
    from concourse.bass_utils import run_bass_kernel_spmd

    if os.environ.get("BASS_TRACE"):
        _install_ntff_shim()

    lead = z.shape[:-1]
    zf = np.ascontiguousarray(np.asarray(z, dtype=np.float32).reshape(-1, D))
    W = np.asarray(masks, dtype=np.float32) * np.asarray(U, dtype=np.float32)
    C_w = np.asarray(C_w, dtype=np.float32)
    C_b = np.asarray(C_b, dtype=np.float32)

    # Detect all-zero 128x128 blocks of W; build per-(degree, rank-tile)
    # contraction ranges. Only provably-zero blocks are skipped.
    blk = (
        np.abs(W.reshape(DEGREE, KT, P, DT, P)).max(axis=(2, 4)) > 0.0
    )  # [i, kt, dt]
    ranges = []
    for i in range(DEGREE):
        row = []
        for kt in range(KT):
            nz = np.flatnonzero(blk[i, kt])
            row.append((int(nz[0]), int(nz[-1])) if len(nz) else None)
        ranges.append(tuple(row))
    ranges = tuple(ranges)

    # Device layouts (see _build for index conventions). Weights and
    # activations go to the device as bf16 (round-to-nearest): the matmuls
    # run at the same 1 row/cycle as fp32r, but DMA traffic, SBUF footprint
    # and DVE element width all halve. Measured end-to-end rel err ~5e-3
    # (PSUM still accumulates fp32), well inside the 2e-2 gate.
    from ml_dtypes import bfloat16

    w_dev = np.ascontiguousarray(
        W.reshape(DEGREE, KT, P, DT, P).transpose(0, 1, 4, 3, 2)
    ).reshape(DEGREE, KT, P, DT * P).astype(bfloat16)
    c_dev = np.ascontiguousarray(
        C_w.reshape(OT, P, KT, P).transpose(0, 3, 2, 1)
    ).reshape(OT, P, KT * P).astype(bfloat16)
    cb_dev = np.ascontiguousarray(C_b.reshape(OT, P).T)

    in_maps = []
    for c in range(N_CORES):
        zs = zf[c * TOK : (c + 1) * TOK]  # [TOK, D]
        z_dev = np.ascontiguousarray(
            zs.T.reshape(DT, P, TOK).transpose(1, 0, 2)
        ).reshape(P, DT * TOK).astype(bfloat16)
        in_maps.append({"z": z_dev, "w": w_dev, "c": c_dev, "cb": cb_dev})

    if _CACHE.get("ranges") != ranges:
        _CACHE["nc"] = _build(ranges)
        _CACHE["ranges"] = ranges
    nc = _CACHE["nc"]

    res = run_bass_kernel_spmd(nc, in_maps, core_ids=list(range(N_CORES)))
    _CACHE["last_result"] = res

    parts = [res.results[c]["x"].T for c in range(N_CORES)]  # each [TOK, O]
    x = np.concatenate(parts, axis=0)
    return x.reshape(*lead, O)
